# revision 46
# baseline (speedup 1.0000x reference)
"""Trainium2 Bass kernel: segmented (expert-parallel) LoRA with dropout.

Computes  out = result + scatter_e( (data_e * keep_e * scale) @ A_e^T @ B_e^T )
where keep = (drop_mask >= 0.05), scale = 2.0 / 0.95, and each of the E=8
adapters owns a contiguous batch segment of 2 batches (4096 tokens).

Sharding: expert-parallel - core e gets adapter e's A/B and its batch segment
(data/drop_mask/result slices), so there are no cross-core collectives.

The kernel is HBM-bound (~100 MB/core after staging: data/mask fp8, res/out
fp16; tolerance 2e-2, measured end-to-end error ~9e-3).

v11: token-split software pipeline with 2-bank GEMM2 gens. Work splits
into token halves tau in {0,1}; phase 1 (dropout + GEMM1, DVE-bound) of
tau=1 overlaps phase 2 (GEMM2 + residual + stores, DMA-bound) of tau=0:

  A: phase1(tau0)                      DVE-paced,  DMA prefetches res(tau0)
  B: phase1(tau1) || phase2(tau0)      DMA-paced   (~full duplex HBM)
  C: phase2(tau1)                      DMA/engine-balanced

The v10 attempt failed (351us) because GEMM2 gens were [128,2048] (4 PSUM
banks): with mids(tau1) occupying the other 4 banks, gens single-buffered
into a ~2.6us/gen MM->ACT serial chain that paced section B and let the
stores/res-loads trail into C. v11 halves the gen to [128,1024] (2 banks,
2 concurrent row-group matmuls), so TWO gens double-buffer inside 4 banks
and the chain pipelines at the ACT drain rate (~1.15us/gen), hidden under
the DMA-bound span.

Carried over from v8.x (all hardware-measured):
  - GEMM1 is M=16: 4x column-tiled (token block b -> col group b), 4
    matmuls concurrent; mids land bank-DISJOINT in PSUM (concurrent
    matmuls sharing a bank corrupt it) at partitions 32j.
  - The mid layout col-tiling produces is exactly the rhs layout the
    row-tiled GEMM2 (K=16) consumes.
  - Dropout stays entirely on DVE (fused STT fp8 at 1x): offloading any
    of it to GpSimd slows the remaining DVE ops ~1.8x (shared SBUF port).
  - Residual adds run at DVE 2x on bf16 stages drained by ACT (a DVE
    PSUM-copy lowers to a 1x CAST - worse); in C, 1/4 of chunks add
    directly from PSUM at 1x to keep ACT under the DMA floor.
  - res prefetch halves (1 MB) are FIFO-interleaved on the SP/ACT HWDGE
    rings between piece loads (a separate res queue steals ~1/3 of the
    bandwidth via packet round-robin and starves the dropout pipeline).
  - DEADLOCK discipline: any WAR-gated DMA trigger is EMITTED only after
    the instructions that free its slot, or it head-of-line-blocks its
    queue against its own dependencies.

Weights are host-packed into the exact SBUF layouts (tiny):
  a_pk[p, c*R+j]    = A[j, c*128+p] * scale  (bf16)  == scaled A^T chunks
  b_tl[32g+j, h]    = B[h, j] for g in 0..3  (bf16)  == B^T replicated at
                      the 4 row-group partition bases
"""

import numpy as np
from contextlib import ExitStack

import ml_dtypes

from concourse import bass, bacc, mybir, tile
from concourse.bass_utils import run_bass_kernel_spmd

# Problem constants (hardcoded per the self-contained-kernel contract).
E = 8
B, S, H, R = 16, 2048, 4096, 16
SEG = B // E
TOK = SEG * S          # tokens per core = 4096
P = 128                # partitions
P_DROP = 0.05
SCALING = 2.0
SCALE = SCALING / (1.0 - P_DROP)

F32 = mybir.dt.float32
F16 = mybir.dt.float16
BF16 = mybir.dt.bfloat16
F8 = mybir.dt.float8e4
BF16_NP = ml_dtypes.bfloat16
F8_NP = ml_dtypes.float8_e4m3   # TRN FP8_EXP4 semantics (inf at S.1111.000)
F16_NP = np.float16

TT = TOK // 2          # tokens per tau half (2048)
CD = 8                 # h chunks per data/mask DMA group (2 MB fp8)
CR = 4                 # h chunks per res/out tile (2 MB fp16)
GD = 32 // CD          # data/mask groups per tau (4)
GR = 32 // CR          # res/out tiles per tau (8)

LAST_RESULTS = None    # BassKernelResults of the most recent run (for test.py)


def build_nc(h=H, r=R, num_devices=E):
    """Build the single-core Bass/Tile program (run SPMD on all cores)."""
    hc = h // P                    # 128-row h chunks (32)

    nc = bacc.Bacc("TRN2", target_bir_lowering=False, debug=False,
                   num_devices=num_devices)

    data = nc.dram_tensor("data", [2, GD, CD, P, TT], F8,
                          kind="ExternalInput").ap()
    mask = nc.dram_tensor("mask", [2, GD, CD, P, TT], F8,
                          kind="ExternalInput").ap()
    res = nc.dram_tensor("res", [2, GR, CR, P, TT], F16,
                         kind="ExternalInput").ap()
    a_pk = nc.dram_tensor("a_pk", [P, hc * r], BF16, kind="ExternalInput").ap()
    b_tl = nc.dram_tensor("b_tl", [P, h], BF16, kind="ExternalInput").ap()
    out = nc.dram_tensor("out", [2, GR, CR, P, TT], F16,
                         kind="ExternalOutput").ap()

    with ExitStack() as ctx:
        tc = ctx.enter_context(tile.TileContext(nc))
        consts = ctx.enter_context(tc.tile_pool(name="consts", bufs=1))
        dpool = ctx.enter_context(tc.tile_pool(name="dpool", bufs=2))
        mpool = ctx.enter_context(tc.tile_pool(name="mpool", bufs=2))
        dropp = ctx.enter_context(tc.tile_pool(name="dropp", bufs=3))
        rpool = ctx.enter_context(tc.tile_pool(name="rpool", bufs=6))
        stpool = ctx.enter_context(tc.tile_pool(name="stpool", bufs=3))
        psM = ctx.enter_context(tc.tile_pool(name="psM", bufs=1, space="PSUM"))
        psO = ctx.enter_context(tc.tile_pool(name="psO", bufs=2, space="PSUM"))

        a_sb = consts.tile([P, hc * r], BF16)
        nc.gpsimd.dma_start(a_sb, a_pk)
        b_sb = consts.tile([P, h], BF16)   # loaded later, off the hot start
        mid_sb = consts.tile([P, 2, 512], BF16)

        res_tiles = {}
        for tau in range(2):
            for k in range(GR):
                if tau == 1 and k >= 6:
                    continue      # (1,6),(1,7) allocated from idle d/m pools
                res_tiles[(tau, k)] = rpool.tile(
                    [P, CR, TT], F16, tag="res", name=f"res_{tau}_{k}")

        def issue_res(tau, k, eng):
            eng.dma_start(res_tiles[(tau, k)],
                          res[tau][k].rearrange("j p t -> p j t"))

        def issue_res_half(tau, k, half, eng):
            hw = CR // 2
            eng.dma_start(
                res_tiles[(tau, k)][:, half * hw:(half + 1) * hw, :],
                res[tau][k][half * hw:(half + 1) * hw].rearrange(
                    "j p t -> p j t"))

        def load_piece(tau, g):
            dt_ = dpool.tile([P, CD, TT], F8, tag="d")
            nc.sync.dma_start(
                dt_, data[tau][g].rearrange("j p t -> p j t"))
            mt = mpool.tile([P, CD, TT], F8, tag="m")
            nc.scalar.dma_start(
                mt, mask[tau][g].rearrange("j p t -> p j t"))
            return dt_, mt

        def load_ramp(tau, g, j0, nj):
            dt_ = dpool.tile([P, nj, TT], F8, tag="d")
            nc.sync.dma_start(
                dt_, data[tau][g][j0:j0 + nj].rearrange("j p t -> p j t"))
            mt = mpool.tile([P, nj, TT], F8, tag="m")
            nc.scalar.dma_start(
                mt, mask[tau][g][j0:j0 + nj].rearrange("j p t -> p j t"))
            return dt_, mt

        def phase1_piece(tau, mids, data_sb, mask_sb, g, j0, nj):
            """Dropout (STT per 2 chunks) + col-tiled GEMM1 gens.
            Tile indices are piece-local; the global chunk is CD*g + j0 + ."""
            for j2 in range(nj // 2):
                drop2 = dropp.tile([P, 2, TT], F8, tag="drop")
                nc.vector.scalar_tensor_tensor(
                    drop2, mask_sb[:, 2 * j2:2 * j2 + 2, :],
                    P_DROP, data_sb[:, 2 * j2:2 * j2 + 2, :],
                    op0=mybir.AluOpType.is_ge, op1=mybir.AluOpType.mult)
                for jj in range(2):
                    c = CD * g + j0 + 2 * j2 + jj
                    for cg in range(4):
                        nc.tensor.matmul(
                            mids[32 * cg:32 * cg + 16, cg, :],
                            lhsT=a_sb[:, bass.ts(c, r)],
                            rhs=drop2[:, jj, bass.ts(cg, 512)],
                            start=(c == 0), stop=(c == hc - 1),
                            tile_position=(0, 32 * cg))

        def drain_mids(tau, mids):
            for cg in range(4):
                nc.scalar.copy(mid_sb[32 * cg:32 * cg + 16, tau, :],
                               mids[32 * cg:32 * cg + 16, cg, :])

        def phase2_group(tau, k, direct_chunks=()):
            """GEMM2 + drain + residual add for one 4-chunk res tile.
            Per chunk: two [128,1024] 2-bank gens (2 concurrent row-group
            matmuls each), double-buffered in psO; ACT drains both into one
            [P,2048] bf16 stage, then ONE DVE 2x add - unless the chunk is
            direct (2 DVE 1x adds straight from PSUM, relieving ACT)."""
            rt = res_tiles[(tau, k)]
            for cc in range(CR):
                c = CR * k + cc
                direct = cc in direct_chunks
                stage = None if direct else stpool.tile([P, TT], BF16,
                                                        tag="st")
                for h2 in range(2):
                    o_ps = psO.tile([P, 1024], F32, tag="ps")
                    for rr in range(2):
                        rg = 2 * h2 + rr
                        nc.tensor.matmul(
                            o_ps[:, bass.ts(rr, 512)],
                            lhsT=b_sb[32 * rg:32 * rg + 16, bass.ts(c, P)],
                            rhs=mid_sb[32 * rg:32 * rg + 16, tau, :],
                            start=True, stop=True, tile_position=(32 * rg, 0))
                    if direct:
                        nc.vector.tensor_add(
                            rt[:, cc, bass.ts(h2, 1024)], o_ps,
                            rt[:, cc, bass.ts(h2, 1024)])
                    else:
                        nc.scalar.copy(stage[:, bass.ts(h2, 1024)], o_ps)
                if not direct:
                    nc.vector.tensor_add(rt[:, cc, :], stage, rt[:, cc, :])
            eng = nc.sync if k % 2 == 0 else nc.gpsimd
            eng.dma_start(out[tau][k].rearrange("j p t -> p j t"), rt)

        # ---- section A: phase1(tau0), res(tau0) prefetch -----------------
        mids0 = psM.tile([P, 4, 512], F32, tag="ps", name="mids0")
        # NOTE measured: steepening this ramp (2,2,4,...) makes the bufs=2
        # lookahead bubbles WORSE (12-17us stalls), and deferring the res
        # halves past the loop queues 9 MB ahead of section B's loads on the
        # same rings (periodic B stalls). This exact schedule measured best.
        piecesA = [(0, 0, 2), (0, 2, 2), (0, 4, 2), (0, 6, 2),
                   (1, 0, 4), (1, 4, 4), (2, 0, 8), (3, 0, 8)]
        nres_half = 0
        for idx, (g, j0, nj) in enumerate(piecesA):
            data_sb, mask_sb = load_ramp(0, g, j0, nj)
            phase1_piece(0, mids0, data_sb, mask_sb, g, j0, nj)
            if idx == 0:
                nc.gpsimd.dma_start(b_sb, b_tl)
            # one 1 MB res half per piece on the alternating load rings,
            # only after the ramp is established (earlier starves the STTs)
            if idx >= 3 and nres_half < 8:
                eng = nc.sync if nres_half % 2 == 0 else nc.scalar
                issue_res_half(0, nres_half // 2, nres_half % 2, eng)
                nres_half += 1
        while nres_half < 8:       # rest of tiles (0, 0..3): fresh slots
            eng = nc.sync if nres_half % 2 == 0 else nc.scalar
            issue_res_half(0, nres_half // 2, nres_half % 2, eng)
            nres_half += 1
        issue_res(0, 4, nc.sync)   # fresh slots, boundary prefetch
        issue_res(0, 5, nc.scalar)

        # ---- section B: phase1(tau1) || phase2(tau0) ---------------------
        drain_mids(0, mids0)
        mids1 = psM.tile([P, 4, 512], F32, tag="ps", name="mids1")
        nxt = load_piece(1, 0)            # loads run one step ahead
        for s in range(GD):
            data_sb, mask_sb = nxt
            if s + 1 < GD:
                nxt = load_piece(1, s + 1)
            phase1_piece(1, mids1, data_sb, mask_sb, s, 0, CD)
            phase2_group(0, 2 * s)
            phase2_group(0, 2 * s + 1)
            if s == 0:
                # slots of (0,0)/(0,1) just freed by groups 0/1. On gpsimd:
                # on the scalar ring these 4 MB queue ahead of the piece-2/3
                # MASK loads (FIFO) and stall the B dropout pipeline ~24us.
                issue_res(0, 6, nc.gpsimd)
                issue_res(0, 7, nc.gpsimd)
            else:
                # slots freed by groups 2s, 2s+1 -> res(1, 2s-2), (1, 2s-1)
                issue_res(1, 2 * s - 2, nc.gpsimd)
                issue_res(1, 2 * s - 1, nc.gpsimd)

        # ---- section C: phase2(tau1) -------------------------------------
        drain_mids(1, mids1)
        for k, pool, tg in ((6, dpool, "d"), (7, mpool, "m")):
            res_tiles[(1, k)] = pool.tile([P, CR, TT], F16, tag=tg,
                                          name=f"res_1_{k}")
        issue_res(1, 6, nc.sync)
        issue_res(1, 7, nc.gpsimd)
        for k in range(GR):
            # every 4th chunk adds directly from PSUM (ACT relief)
            phase2_group(1, k, direct_chunks=(3,) if k % 2 == 1 else ())
    nc.compile()
    return nc


def pack_weights(lora_a, lora_b, h=H, r=R):
    """Pack A (pre-scaled) and B into the SBUF layouts the kernel expects."""
    e = lora_a.shape[0]
    hc = h // P
    a_sc = (np.asarray(lora_a, np.float32) * SCALE).astype(BF16_NP)   # (E,R,H)
    a_pk = np.ascontiguousarray(
        a_sc.reshape(e, r, hc, P).transpose(0, 3, 2, 1)).reshape(e, P, hc * r)
    b_t = np.ascontiguousarray(
        np.asarray(lora_b, np.float32).astype(BF16_NP).transpose(0, 2, 1))
    b_tl = np.zeros((e, P, h), BF16_NP)
    for g in range(4):
        b_tl[:, 32 * g:32 * g + r, :] = b_t
    return a_pk, b_tl


def kernel(result, data, drop_mask, lora_a, lora_b, _trace=False):
    global LAST_RESULTS
    result = np.asarray(result, np.float32)
    data = np.asarray(data, np.float32)
    drop_mask = np.asarray(drop_mask, np.float32)
    hc = H // P

    # per-core slices, transposed to [H, tok] (hidden on partitions), staged
    # in stream dtype, then split into token halves [tau, group, chunk, P, TT]
    def stage_dm(x):
        xt = np.ascontiguousarray(
            x.reshape(E, TOK, H).astype(F8_NP).transpose(0, 2, 1))
        xt = xt.reshape(E, hc, P, 2, TT).transpose(0, 3, 1, 2, 4)
        return np.ascontiguousarray(xt.reshape(E, 2, GD, CD, P, TT))

    data_t = stage_dm(data)
    mask_t = stage_dm(drop_mask)
    res_t = np.ascontiguousarray(
        result.reshape(E, TOK, H).astype(F16_NP).transpose(0, 2, 1))
    res_t = res_t.reshape(E, hc, P, 2, TT).transpose(0, 3, 1, 2, 4)
    res_t = np.ascontiguousarray(res_t.reshape(E, 2, GR, CR, P, TT))
    a_pk, b_tl = pack_weights(lora_a, lora_b)

    nc = build_nc()
    in_maps = [
        {"data": data_t[e], "mask": mask_t[e], "res": res_t[e],
         "a_pk": a_pk[e], "b_tl": b_tl[e]}
        for e in range(E)
    ]
    LAST_RESULTS = run_bass_kernel_spmd(
        nc, in_maps, core_ids=list(range(E)), trace=_trace)
    out_t = np.stack([LAST_RESULTS.results[e]["out"] for e in range(E)])
    # [E, 2, GR, CR, P, TT] -> [E, H, TOK] -> [B, S, H]
    out_t = out_t.reshape(E, 2, hc, P, TT).transpose(0, 2, 3, 1, 4)
    out_t = out_t.reshape(E, H, TOK).astype(np.float32)
    return np.ascontiguousarray(out_t.transpose(0, 2, 1)).reshape(B, S, H)


if __name__ == "__main__":
    rng = np.random.default_rng(0)
    inputs = {
        "result": rng.standard_normal((B, S, H), dtype=np.float32),
        "data": rng.standard_normal((B, S, H), dtype=np.float32),
        "drop_mask": rng.random((B, S, H), dtype=np.float32),
        "lora_a": (rng.standard_normal((E, R, H), dtype=np.float32) * 0.02),
        "lora_b": (rng.standard_normal((E, H, R), dtype=np.float32) * 0.02),
    }
    out = kernel(**inputs)
    print("out", out.shape, out.dtype)
